# revision 8
# baseline (speedup 1.0000x reference)
"""Bahdanau attention kernel for 8 TRN2 NeuronCores.

Math: scores[q,k] = w2 . tanh(qW[q,:] + kW[k,:] + b1) (+ b2, dropped: softmax
is shift-invariant). The tanh over the [B,Q,K,A] tensor is replaced by a
separable product expansion fitted offline:

    tanh(x + y) ~= sum_j F_j(x) * psi_j(y),   F_j = sum_i C_ij phi_i(x)

The x-side combined functions F_j are folded on DVE/Pool so the TensorEngine
does one contraction group per y-function (4 matmuls each) instead of one per
(i,j) pair. b1 is folded into the kW matmul as an extra rank-1 contraction
chunk. Factor activations read qW/kW straight from PSUM; tanh+sin live in one
HW activation table (silu_and_others) so there is a single table load.
Softmax runs per k-half (flash-style within core) so exp/mask/transpose/
context overlap the tail of the score matmuls; masking is a -30 additive
pre-exp term; no max subtraction (scores are bounded).

Sharding: data-parallel, core = (batch b, query-half qh); each core computes
a [128, 512] block of weights and context. Output: (context, weights).
"""

import numpy as np
import ml_dtypes

from contextlib import ExitStack
from concourse import bass, bacc, tile, mybir
from concourse.bass_utils import run_bass_kernel_spmd

BF16 = mybir.dt.bfloat16
F32 = mybir.dt.float32
AF = mybir.ActivationFunctionType
OP = mybir.AluOpType
NPBF = ml_dtypes.bfloat16

B, Q, K, H, A = 4, 256, 512, 512, 512
QSH = 128
N_CORES = 8
PH = float(np.pi / 4)
TMAX = 3.2          # |spline arg| budget for Sin
XMAX = 2.16         # value range bound of x / y
ALPHA = 1.5
MASK_NEG = -30.0

# ---- factor model (same fitted atoms/pairs as the 23-pair baseline) -------
# atom spec: ('one',) | ('lin',) | ('tanh', alpha, mu) | ('sin', w, sgn)
TANH_MUS = [-1.35, -0.9, -0.45, 0.0, 0.45, 0.9, 1.35]
WSIN = 2.748893571891069
XATOMS = [('one',)] + [('tanh', ALPHA, m) for m in TANH_MUS] + \
         [('sin', WSIN, 1.0), ('sin', WSIN, -1.0)]
YATOMS = [('lin',)] + [('tanh', ALPHA, m) for m in TANH_MUS] + \
         [('sin', WSIN, 1.0), ('sin', WSIN, -1.0)]
PAIRS = [(0, 0, 0.055989194052271596), (0, 4, 0.0464876907294621),
         (4, 3, 0.3586833482863322), (5, 4, -0.08820327379751021),
         (3, 4, 0.09356806623204295), (8, 9, 0.018391745760703182),
         (9, 8, -0.018507904727926565), (4, 5, -0.36256935752521474),
         (8, 8, 0.02378481035147289), (9, 9, -0.024117013703251228),
         (2, 7, -0.41200786381688864), (7, 2, -0.34507666694970107),
         (1, 6, 0.3489870893514301), (6, 1, 0.40992867906619745),
         (0, 3, -0.008027778964429386), (0, 1, 0.3522094340996041),
         (3, 6, -0.1562301094680913), (5, 2, 0.15939699613235894),
         (0, 7, 0.35697948888882985), (6, 3, -0.31773023083570284),
         (2, 5, 0.3118364963378), (0, 6, -0.16999293064737495),
         (0, 2, -0.1598292790689563)]

PROBES = [(AF.Tanh, 'tanh'), (AF.Sin, 'sin'), (AF.Square, 'square'),
          (AF.Relu, 'relu'), (AF.Silu, 'silu'), (AF.Exp, 'exp'),
          (AF.Copy, 'copy'), (AF.Gelu, 'gelu')]
PROBE_LO, PROBE_HI, PROBE_N = -4.5, 4.5, 128 * 512
EMIT_PROBES = True


def _trig_clip(w):
    c = (TMAX - PH) / w
    return c if c < XMAX else None


def _atom_bias_col(spec, consts_cols):
    """Column index in the consts tile for this atom's ACT bias."""
    key = None
    if spec[0] == 'tanh':
        key = ('t', spec[1], spec[2])
    elif spec[0] == 'sin':
        key = ('s', spec[2])
    return consts_cols[key]


def _consts_layout():
    """All ACT bias constants, one column each (broadcast over partitions)."""
    cols = {('z',): 0}
    vals = [0.0]
    for spec in XATOMS + YATOMS:
        if spec[0] == 'tanh':
            key = ('t', spec[1], spec[2])
            if key not in cols:
                cols[key] = len(vals)
                vals.append(-spec[1] * spec[2])
        elif spec[0] == 'sin':
            key = ('s', spec[2])
            if key not in cols:
                cols[key] = len(vals)
                vals.append(PH * spec[2])
    return cols, vals


CONSTS_COLS, CONSTS_VALS = _consts_layout()
NCONSTS = len(CONSTS_VALS)

# group pairs by y-atom: j -> [(i, c), ...]
YGROUPS = {}
for (xi, yi, cf) in PAIRS:
    YGROUPS.setdefault(yi, []).append((xi, cf))
# x-atom emission order = index order; sort each group's terms by x readiness
for yi in YGROUPS:
    YGROUPS[yi].sort(key=lambda t: t[0])
USED_X = sorted({p[0] for p in PAIRS})
# y emission order: tanh atoms first, then sin (needs clip), lin is on DVE
Y_TANH = [j for j, s in enumerate(YATOMS) if s[0] == 'tanh' and j in YGROUPS]
Y_SIN = [j for j, s in enumerate(YATOMS) if s[0] == 'sin' and j in YGROUPS]
Y_LIN = [j for j, s in enumerate(YATOMS) if s[0] == 'lin' and j in YGROUPS]
# PE consumption order: first tanh, then lin (DVE), remaining tanhs, sins
if Y_TANH:
    Y_ORDER = [Y_TANH[0]] + Y_LIN + Y_TANH[1:] + Y_SIN
else:
    Y_ORDER = Y_LIN + Y_SIN


def _build_kernel():
    nc = bacc.Bacc("TRN2", target_bir_lowering=False, debug=False,
                   num_devices=N_CORES)

    d_qt = nc.declare_dram_parameter("qt", [H, QSH], BF16, isOutput=False)
    d_kt = nc.declare_dram_parameter("kt", [H, K], BF16, isOutput=False)
    d_v = nc.declare_dram_parameter("v", [K, H], BF16, isOutput=False)
    d_m = nc.declare_dram_parameter("m", [QSH, K], BF16, isOutput=False)
    d_w1 = nc.declare_dram_parameter("w1", [2 * H, A], BF16, isOutput=False)
    d_b1r = nc.declare_dram_parameter("b1r", [1, A], BF16, isOutput=False)
    d_ones = nc.declare_dram_parameter("onesr", [1, K], BF16, isOutput=False)
    d_w2c = nc.declare_dram_parameter("w2c", [128, 4], F32, isOutput=False)
    d_w2bc = nc.declare_dram_parameter("w2bc", [128, 512], BF16, isOutput=False)
    d_cb = nc.declare_dram_parameter("consts", [128, NCONSTS], F32,
                                     isOutput=False)
    d_id = nc.declare_dram_parameter("ident", [128, 128], BF16, isOutput=False)
    d_wout = nc.declare_dram_parameter("wout", [QSH, K], F32, isOutput=True)
    d_cout = nc.declare_dram_parameter("cout", [QSH, H], F32, isOutput=True)
    if EMIT_PROBES:
        d_pin = nc.declare_dram_parameter("probe_in", [128, 512], F32,
                                          isOutput=False)
        d_pout = nc.declare_dram_parameter(
            "probe_out", [128, 512 * len(PROBES)], F32, isOutput=True)

    with tile.TileContext(nc) as tc, ExitStack() as ctx:
        sb = ctx.enter_context(tc.tile_pool(name="sb", bufs=1))
        ps_sc = ctx.enter_context(tc.tile_pool(name="pssc", bufs=1,
                                               space="PSUM"))
        pre_ctx = ExitStack()
        ps_pre = pre_ctx.enter_context(tc.tile_pool(name="pspre", bufs=1,
                                                    space="PSUM"))

        # ---- input DMAs ------------------------------------------------
        # critical path (gpsimd queue, cheap dispatch): w1A, qT, w1B, kT
        w1bA = sb.tile([128, 4 * A], BF16, tag="w1bA")
        w1bB = sb.tile([128, 4 * A], BF16, tag="w1bB")
        qTs = sb.tile([128, 4 * QSH], BF16, tag="qTs")
        kTs = sb.tile([128, 4 * K], BF16, tag="kTs")
        for hc in range(4):
            nc.gpsimd.dma_start(w1bA[:, hc * A:(hc + 1) * A],
                                d_w1[hc * 128:(hc + 1) * 128, :])
        for hc in range(4):
            nc.gpsimd.dma_start(qTs[:, hc * QSH:(hc + 1) * QSH],
                                d_qt[hc * 128:(hc + 1) * 128, :])
        for hc in range(4):
            nc.gpsimd.dma_start(w1bB[:, hc * A:(hc + 1) * A],
                                d_w1[(4 + hc) * 128:(5 + hc) * 128, :])
        for hc in range(4):
            nc.gpsimd.dma_start(kTs[:, hc * K:(hc + 1) * K],
                                d_kt[hc * 128:(hc + 1) * 128, :])
        # non-critical (sync queue): consts, w2, b1 row, values, mask, ident
        cb = sb.tile([128, NCONSTS], F32, tag="cb")
        nc.sync.dma_start(cb[:], d_cb[:])
        w2c = sb.tile([128, 4], F32, tag="w2c")
        nc.sync.dma_start(w2c[:], d_w2c[:])
        w2bc = sb.tile([128, 512], BF16, tag="w2bc")
        nc.sync.dma_start(w2bc[:], d_w2bc[:])
        b1r = sb.tile([1, A], BF16, tag="b1r")
        nc.sync.dma_start(b1r[:], d_b1r[:])
        onesr = sb.tile([1, K], BF16, tag="onesr")
        nc.sync.dma_start(onesr[:], d_ones[:])
        vb = sb.tile([128, 4 * H], BF16, tag="vb")
        for kc in range(4):
            nc.sync.dma_start(vb[:, kc * H:(kc + 1) * H],
                              d_v[kc * 128:(kc + 1) * 128, :])
        mf = sb.tile([128, K], BF16, tag="mf")
        nc.sync.dma_start(mf[:], d_m[:])
        ident = sb.tile([128, 128], BF16, tag="ident")
        nc.sync.dma_start(ident[:], d_id[:])
        if EMIT_PROBES:
            pin = sb.tile([128, 512], F32, tag="pin")
            nc.sync.dma_start(pin[:], d_pin[:])

        # ---- qWT [a, q] in PSUM ----------------------------------------
        qwt_ps = ps_pre.tile([128, 512], F32, tag="qwt")
        for ab in range(4):
            for hc in range(4):
                nc.tensor.matmul(
                    qwt_ps[:, ab * 128:(ab + 1) * 128],
                    w1bA[:, hc * A + ab * 128: hc * A + (ab + 1) * 128],
                    qTs[:, hc * 128:(hc + 1) * 128],
                    start=(hc == 0), stop=(hc == 3))

        # ---- kWT [a, k] + b1 (extra rank-1 chunk) in PSUM ---------------
        kwt_ps = ps_pre.tile([128, 2048], F32, tag="kwt")
        for ab in range(4):
            for hc in range(4):
                nc.tensor.matmul(
                    kwt_ps[:, ab * 512:(ab + 1) * 512],
                    w1bB[:, hc * A + ab * 128: hc * A + (ab + 1) * 128],
                    kTs[:, hc * 512:(hc + 1) * 512],
                    start=(hc == 0), stop=False)
            nc.tensor.matmul(
                kwt_ps[:, ab * 512:(ab + 1) * 512],
                b1r[:, ab * 128:(ab + 1) * 128],
                onesr[:],
                start=False, stop=True)

        # ---- x-side atoms (ACT from PSUM) + w2-fold (Pool) --------------
        def emit_atom(engine_src, spec, out, clip_cache, clip_pool, width):
            kind = spec[0]
            if kind == 'tanh':
                nc.scalar.activation(
                    out[:], engine_src[:], AF.Tanh,
                    bias=cb[:, _atom_bias_col(spec, CONSTS_COLS):
                            _atom_bias_col(spec, CONSTS_COLS) + 1],
                    scale=float(spec[1]))
            elif kind == 'sin':
                w = spec[1]
                c = _trig_clip(w)
                if c is None:
                    src = engine_src
                else:
                    if 'clip' not in clip_cache:
                        ct = sb.tile([128, width], F32,
                                     tag=f"clip{width}")
                        clip_pool(ct[:], engine_src[:], float(c), float(-c),
                                  OP.min, OP.max)
                        clip_cache['clip'] = ct
                    src = clip_cache['clip']
                nc.scalar.activation(
                    out[:], src[:], AF.Sin,
                    bias=cb[:, _atom_bias_col(spec, CONSTS_COLS):
                            _atom_bias_col(spec, CONSTS_COLS) + 1],
                    scale=float(w))

        xph = {}      # raw x atom tiles (bf16)
        xclip_cache = {}
        # emit tanh x-atoms, then sin x-atoms (clip on DVE)
        x_order = ([i for i in USED_X if XATOMS[i][0] == 'tanh'] +
                   [i for i in USED_X if XATOMS[i][0] == 'sin'])
        for i in x_order:
            t = sb.tile([128, 512], BF16, tag=f"xf{i}")
            emit_atom(qwt_ps, XATOMS[i], t, xclip_cache,
                      nc.vector.tensor_scalar, 512)
            xph[i] = t

        # w2-fold x atoms on Pool (4 per-partition-scalar passes each)
        xw = {}
        for i in USED_X:
            if XATOMS[i][0] == 'one':
                xw[i] = w2bc
                continue
            t = sb.tile([128, 512], BF16, tag=f"xw{i}")
            for ab in range(4):
                sl = slice(ab * 128, (ab + 1) * 128)
                nc.gpsimd.tensor_scalar_mul(t[:, sl], xph[i][:, sl],
                                            w2c[:, ab:ab + 1])
            xw[i] = t

        # ---- y lin atom (DVE copy from PSUM) ----------------------------
        yt = {}
        yclip_cache = {}
        for j in Y_LIN:
            t = sb.tile([128, 2048], BF16, tag=f"yf{j}")
            nc.vector.tensor_copy(t[:], kwt_ps[:])
            yt[j] = t

        # ---- y tanh atoms (ACT from PSUM) -------------------------------
        for j in Y_TANH:
            t = sb.tile([128, 2048], BF16, tag=f"yf{j}")
            emit_atom(kwt_ps, YATOMS[j], t, yclip_cache,
                      nc.vector.tensor_scalar, 2048)
            yt[j] = t

        # ---- F_j chains (x-only deps; alternate Pool/DVE) ---------------
        fts = {}
        for n, j in enumerate(Y_ORDER):
            terms = YGROUPS[j]
            ft = sb.tile([128, 512], BF16, tag=f"F{j}")
            i0, c0 = terms[0]
            nc.vector.tensor_scalar_mul(ft[:], xw[i0][:], float(c0))
            for (ii, cc) in terms[1:]:
                nc.vector.scalar_tensor_tensor(ft[:], xw[ii][:], float(cc),
                                               ft[:], OP.mult, OP.add)
            fts[j] = ft

        # ---- y sin atoms (clip on DVE, late; ACT reads clipped SBUF) ----
        for j in Y_SIN:
            t = sb.tile([128, 2048], BF16, tag=f"yf{j}")
            emit_atom(kwt_ps, YATOMS[j], t, yclip_cache,
                      nc.vector.tensor_scalar, 2048)
            yt[j] = t

        # qwt/kwt PSUM banks are dead past this point; free them for the tail
        pre_ctx.close()
        ps_tail = ctx.enter_context(tc.tile_pool(name="pstail", bufs=1,
                                                 space="PSUM"))
        ps_tp = ctx.enter_context(tc.tile_pool(name="pstp", bufs=2,
                                               space="PSUM"))

        # ---- score matmuls: per y-atom, k-half A then k-half B ----------
        # each half padded to a full PSUM bank (no read-while-accumulate
        # sharing between the halves)
        sc_A_full = ps_sc.tile([128, 512], F32, tag="scA")
        sc_B_full = ps_sc.tile([128, 512], F32, tag="scB")
        sc_A = sc_A_full[:, 0:256]
        sc_B = sc_B_full[:, 0:256]
        nj = len(Y_ORDER)
        for n, j in enumerate(Y_ORDER):
            for half, sc in ((0, sc_A), (1, sc_B)):
                for ab in range(4):
                    nc.tensor.matmul(
                        sc[:],
                        fts[j][:, ab * 128:(ab + 1) * 128],
                        yt[j][:, ab * 512 + half * 256:
                              ab * 512 + half * 256 + 256],
                        start=(n == 0 and ab == 0),
                        stop=(n == nj - 1 and ab == 3))

        # exp table prefetch while score matmuls drain
        dummy = sb.tile([128, 1], F32, tag="dummy")
        nc.scalar.activation(dummy[:], cb[:, 0:1], AF.Exp, bias=0.0, scale=1.0)

        # ---- per-half masked softmax + context --------------------------
        ctx_ps = ps_tail.tile([128, 512], F32, tag="ctx")
        ssum = {}
        wexp = {}
        for half, sc in ((0, sc_A), (1, sc_B)):
            sm = sb.tile([128, 256], F32, tag=f"sm{half}")
            nc.vector.scalar_tensor_tensor(sm[:], mf[:, half * 256:
                                                     half * 256 + 256],
                                           MASK_NEG, sc[:], OP.mult, OP.add)
            we = sb.tile([128, 256], BF16, tag=f"we{half}")
            ss = sb.tile([128, 1], F32, tag=f"ss{half}")
            nc.scalar.activation(we[:], sm[:], AF.Exp, bias=0.0, scale=1.0,
                                 accum_out=ss[:])
            wexp[half] = we
            ssum[half] = ss
            # transpose this half and accumulate its context contribution
            wT = sb.tile([128, 256], BF16, tag=f"wT{half}")
            for i in range(2):
                pt = ps_tp.tile([128, 128], BF16, tag="tp")
                nc.tensor.transpose(pt[:], we[:, i * 128:(i + 1) * 128],
                                    ident[:])
                nc.vector.tensor_copy(wT[:, i * 128:(i + 1) * 128], pt[:])
            for i in range(2):
                kc = half * 2 + i
                nc.tensor.matmul(ctx_ps[:], wT[:, i * 128:(i + 1) * 128],
                                 vb[:, kc * 512:(kc + 1) * 512],
                                 start=(kc == 0), stop=(kc == 3))

        stot = sb.tile([128, 1], F32, tag="stot")
        nc.vector.tensor_add(stot[:], ssum[0][:], ssum[1][:])
        rinv = sb.tile([128, 1], F32, tag="rinv")
        nc.vector.reciprocal(rinv[:], stot[:])
        wout = sb.tile([128, 512], F32, tag="wout")
        for half in (0, 1):
            nc.vector.tensor_scalar_mul(wout[:, half * 256:half * 256 + 256],
                                        wexp[half][:], rinv[:])
            nc.sync.dma_start(d_wout[:, half * 256:half * 256 + 256],
                              wout[:, half * 256:half * 256 + 256])
        cout = sb.tile([128, 512], F32, tag="cout")
        nc.vector.tensor_scalar_mul(cout[:], ctx_ps[:], rinv[:])
        nc.sync.dma_start(d_cout[:], cout[:])

        # ---- HW activation probes (temporary, measured offline) ---------
        if EMIT_PROBES:
            pout = sb.tile([128, 512 * len(PROBES)], F32, tag="pout")
            for n, (func, _) in enumerate(PROBES):
                nc.scalar.activation(pout[:, n * 512:(n + 1) * 512],
                                     pin[:], func, bias=0.0, scale=1.0)
            nc.sync.dma_start(d_pout[:], pout[:])

    nc.compile()
    return nc


_NC_CACHE = None


def _get_nc():
    global _NC_CACHE
    if _NC_CACHE is None:
        _NC_CACHE = _build_kernel()
    return _NC_CACHE


def _host_inputs(query, keys, values, mask, W1, b1, w2, b2):
    query = np.asarray(query, np.float32).astype(NPBF)
    keys = np.asarray(keys, np.float32).astype(NPBF)
    values = np.asarray(values, np.float32).astype(NPBF)
    maskb = np.asarray(mask).astype(NPBF)
    W1 = np.ascontiguousarray(np.asarray(W1, np.float32).astype(NPBF))
    b1 = np.asarray(b1, np.float32)
    w2 = np.asarray(w2, np.float32)
    b1r = np.ascontiguousarray(b1.astype(NPBF).reshape(1, A))
    onesr = np.ones((1, K), dtype=NPBF)
    w2cc = np.ascontiguousarray(w2.reshape(4, 128).T.astype(np.float32))
    w2bc = np.ascontiguousarray(
        np.repeat(w2cc.astype(NPBF)[:, :, None], 128, axis=2).reshape(128, 512))
    consts = np.zeros((128, NCONSTS), np.float32)
    for c, v in enumerate(CONSTS_VALS):
        consts[:, c] = v
    ident = np.eye(128, dtype=NPBF)
    probe_in = np.linspace(PROBE_LO, PROBE_HI, PROBE_N, endpoint=False,
                           dtype=np.float32).reshape(128, 512)

    in_maps = []
    for c in range(N_CORES):
        b, qh = c // 2, c % 2
        im = {
            "qt": np.ascontiguousarray(query[b, qh * QSH:(qh + 1) * QSH, :].T),
            "kt": np.ascontiguousarray(keys[b].T),
            "v": np.ascontiguousarray(values[b]),
            "m": np.ascontiguousarray(maskb[b, qh * QSH:(qh + 1) * QSH, :]),
            "w1": W1,
            "b1r": b1r,
            "onesr": onesr,
            "w2c": w2cc,
            "w2bc": w2bc,
            "consts": consts,
            "ident": ident,
        }
        if EMIT_PROBES:
            im["probe_in"] = probe_in
        in_maps.append(im)
    return in_maps


def _run(inputs, trace=False, **kw):
    nc = _get_nc()
    in_maps = _host_inputs(**inputs)
    res = run_bass_kernel_spmd(nc, in_maps, list(range(N_CORES)),
                               trace=trace, **kw)
    context = np.zeros((B, Q, H), np.float32)
    weights = np.zeros((B, Q, K), np.float32)
    for c in range(N_CORES):
        b, qh = c // 2, c % 2
        weights[b, qh * QSH:(qh + 1) * QSH, :] = res.results[c]["wout"]
        context[b, qh * QSH:(qh + 1) * QSH, :] = res.results[c]["cout"]
    return (context, weights), res


def kernel(query, keys, values, mask, W1, b1, w2, b2):
    (context, weights), _ = _run(dict(query=query, keys=keys, values=values,
                                      mask=mask, W1=W1, b1=b1, w2=w2, b2=b2))
    return context, weights


# revision 12
# speedup vs baseline: 2.1093x; 2.1093x over previous
"""Bahdanau attention kernel for 8 TRN2 NeuronCores.

Math: scores[q,k] = w2 . tanh(qW[q,:] + kW[k,:] + b1) (+ b2, dropped: softmax
is shift-invariant). The tanh over the [B,Q,K,A] tensor is replaced by a
separable product expansion fitted offline:

    tanh(x + y) ~= sum_j F_j(x) * psi_j(y),   F_j = w2 * sum_i C_ij phi_i(x)

The x-side combined functions F_j are folded on DVE (one op per nonzero C
entry + one w2-broadcast multiply) so the TensorEngine runs one contraction
group per y-function instead of one per (i,j) pair. b1 is folded into the kW
matmul as an extra rank-1 contraction chunk. Factor activations read qW/kW
straight from PSUM; tanh+sin live in one HW activation table
(silu_and_others) so there is a single table load (+1 for the final exp,
prefetched under the score matmuls). Softmax runs per k-half so
exp/mask/transpose/context overlap the score-matmul tail; masking is a -30
additive pre-exp term; no max subtraction (scores are bounded).

Sharding: data-parallel, core = (batch b, query-half qh); each core computes
a [128, 512] block of weights and context. Output: (context, weights).
"""

import numpy as np
import ml_dtypes

from contextlib import ExitStack
from concourse import bass, bacc, tile, mybir
from concourse.bass_utils import run_bass_kernel_spmd

BF16 = mybir.dt.bfloat16
F32 = mybir.dt.float32
AF = mybir.ActivationFunctionType
OP = mybir.AluOpType
NPBF = ml_dtypes.bfloat16

B, Q, K, H, A = 4, 256, 512, 512, 512
QSH = 128
N_CORES = 8
PH = float(np.pi / 4)
TMAX = 3.2          # |spline arg| budget for Sin
ALPHA = 1.5
MASK_NEG = -30.0

# ---- factor model (fitted offline; see fit.py / fit_run.py) ---------------
# atom spec: ('one',) | ('lin',) | ('tanh', a, mu) | ('sin', w, sgn)
#            | ('silu', a, mu) | ('relu', a, mu) | ('square', a, mu)
# J=5 y-atoms (lin + 4 tanh), 6 used x-atoms, 14 nonzero C entries;
# fitted against measured HW activation profiles, validated end-to-end
# in numpy (incl bf16 fold effects): weights 3.6e-3 / context 4.2e-3.
XATOMS = [('one',), ('sin', 1.6, -1.0), ('sin', 1.6, 1.0),
          ('tanh', 2.1, -0.9), ('tanh', 1.75, -0.6), ('square', 1.0, 0.0),
          ('tanh', 2.1, 0.6), ('sin', 1.1, -1.0), ('tanh', 1.75, 0.3),
          ('sin', 1.1, 1.0)]
YATOMS = [('lin',), ('tanh', 1.0, 0.0), ('tanh', 1.0, 0.3),
          ('tanh', 1.0, -0.9), ('tanh', 1.0, 0.9)]
PAIRS = [
    (0, 0, 0.33883327733454804),
    (2, 0, -0.38084003064057803),
    (2, 1, 0.7521735867138226),
    (3, 1, -0.6526452224841515),
    (3, 2, 0.5507454556928899),
    (3, 3, 0.47094649442556524),
    (6, 1, -3.2655336437209534),
    (6, 2, 3.142604848671578),
    (6, 3, 0.4071331546862096),
    (7, 0, 0.23064867566520686),
    (7, 4, -0.7300622210776363),
    (8, 1, 3.448561459703488),
    (8, 2, -4.25558442481575),
    (8, 4, 0.6800733993698169),
]
XMAX = 2.16


def _trig_clip(w):
    c = (TMAX - PH) / w
    return c if c < XMAX else None


def _consts_layout():
    cols = {('z',): 0}
    vals = [0.0]
    for spec in XATOMS + YATOMS:
        key = None
        bias = None
        if spec[0] in ('tanh', 'silu', 'relu', 'square'):
            key = (spec[0], spec[1], spec[2])
            bias = -spec[1] * spec[2]
        elif spec[0] == 'sin':
            key = ('s', spec[2])
            bias = PH * spec[2]
        if key is not None and key not in cols:
            cols[key] = len(vals)
            vals.append(bias)
    return cols, vals


CONSTS_COLS, CONSTS_VALS = _consts_layout()
NCONSTS = len(CONSTS_VALS)

AF_OF = {'tanh': AF.Tanh, 'sin': AF.Sin, 'silu': AF.Silu, 'relu': AF.Relu,
         'square': AF.Square}

# group pairs by y-atom: j -> [(i, c), ...]; 'one' terms handled at fold time
YGROUPS = {}
for (xi, yi, cf) in PAIRS:
    YGROUPS.setdefault(yi, []).append((xi, cf))
for yi in YGROUPS:
    YGROUPS[yi].sort(key=lambda t: t[0])
USED_X = sorted({p[0] for p in PAIRS})
Y_TANH = [j for j, s in enumerate(YATOMS)
          if s[0] in ('tanh', 'silu', 'relu', 'square') and j in YGROUPS]
Y_SIN = [j for j, s in enumerate(YATOMS) if s[0] == 'sin' and j in YGROUPS]
Y_LIN = [j for j, s in enumerate(YATOMS) if s[0] == 'lin' and j in YGROUPS]
if Y_TANH:
    Y_ORDER = [Y_TANH[0]] + Y_LIN + Y_TANH[1:] + Y_SIN
else:
    Y_ORDER = Y_LIN + Y_SIN


def _build_kernel():
    nc = bacc.Bacc("TRN2", target_bir_lowering=False, debug=False,
                   num_devices=N_CORES)

    d_qt = nc.declare_dram_parameter("qt", [H, QSH], BF16, isOutput=False)
    d_kt = nc.declare_dram_parameter("kt", [H, K], BF16, isOutput=False)
    d_v = nc.declare_dram_parameter("v", [K, H], BF16, isOutput=False)
    d_m = nc.declare_dram_parameter("m", [QSH, K], BF16, isOutput=False)
    d_w1 = nc.declare_dram_parameter("w1", [2 * H, A], BF16, isOutput=False)
    d_b1r = nc.declare_dram_parameter("b1r", [1, A], BF16, isOutput=False)
    d_ones = nc.declare_dram_parameter("onesr", [1, K], BF16, isOutput=False)
    d_w2bc = nc.declare_dram_parameter("w2bc", [128, 512], BF16, isOutput=False)
    d_cb = nc.declare_dram_parameter("consts", [128, NCONSTS], F32,
                                     isOutput=False)
    d_id = nc.declare_dram_parameter("ident", [128, 128], BF16, isOutput=False)
    d_wout = nc.declare_dram_parameter("wout", [QSH, K], F32, isOutput=True)
    d_cout = nc.declare_dram_parameter("cout", [QSH, H], F32, isOutput=True)

    with tile.TileContext(nc) as tc, ExitStack() as ctx:
        sb = ctx.enter_context(tc.tile_pool(name="sb", bufs=1))
        ps_sc = ctx.enter_context(tc.tile_pool(name="pssc", bufs=1,
                                               space="PSUM"))
        pre_ctx = ExitStack()
        ps_pre = pre_ctx.enter_context(tc.tile_pool(name="pspre", bufs=1,
                                                    space="PSUM"))

        # ---- input DMAs: one rearranged DMA per tensor ------------------
        # sync queue: qWT path first, then kWT path; gpsimd: the rest
        w1bA = sb.tile([128, 4 * A], BF16, tag="w1bA")
        w1bB = sb.tile([128, 4 * A], BF16, tag="w1bB")
        qTs = sb.tile([128, 4 * QSH], BF16, tag="qTs")
        kTs = sb.tile([128, 4 * K], BF16, tag="kTs")
        cb = sb.tile([128, NCONSTS], F32, tag="cb")
        nc.sync.dma_start(cb[:], d_cb[:])
        nc.sync.dma_start(w1bA[:].rearrange("p (hc c) -> p hc c", hc=4),
                          d_w1[0:512, :].rearrange("(hc p) c -> p hc c", hc=4))
        nc.sync.dma_start(qTs[:].rearrange("p (hc c) -> p hc c", hc=4),
                          d_qt[:].rearrange("(hc p) c -> p hc c", hc=4))
        nc.scalar.dma_start(w1bB[:].rearrange("p (hc c) -> p hc c", hc=4),
                            d_w1[512:1024, :].rearrange("(hc p) c -> p hc c",
                                                        hc=4))
        nc.gpsimd.dma_start(kTs[:].rearrange("p (hc c) -> p hc c", hc=4),
                            d_kt[:].rearrange("(hc p) c -> p hc c", hc=4))
        b1r = sb.tile([1, A], BF16, tag="b1r")
        nc.gpsimd.dma_start(b1r[:], d_b1r[:])
        onesr = sb.tile([1, K], BF16, tag="onesr")
        nc.gpsimd.dma_start(onesr[:], d_ones[:])
        w2bc = sb.tile([128, 512], BF16, tag="w2bc")
        nc.gpsimd.dma_start(w2bc[:], d_w2bc[:])
        vb = sb.tile([128, 4 * H], BF16, tag="vb")
        nc.gpsimd.dma_start(vb[:].rearrange("p (kc c) -> p kc c", kc=4),
                            d_v[:].rearrange("(kc p) c -> p kc c", kc=4))
        mf = sb.tile([128, K], BF16, tag="mf")
        nc.gpsimd.dma_start(mf[:], d_m[:])
        ident = sb.tile([128, 128], BF16, tag="ident")
        nc.gpsimd.dma_start(ident[:], d_id[:])

        # ---- qWT [a, q] in PSUM (ab-outer: sequential PSUM acc groups) --
        qwt_ps = ps_pre.tile([128, 512], F32, tag="qwt")
        for ab in range(4):
            for hc in range(4):
                nc.tensor.matmul(
                    qwt_ps[:, ab * 128:(ab + 1) * 128],
                    w1bA[:, hc * A + ab * 128: hc * A + (ab + 1) * 128],
                    qTs[:, hc * 128:(hc + 1) * 128],
                    start=(hc == 0), stop=(hc == 3))

        # ---- kWT [a, k] + b1 (extra rank-1 chunk closes each group) -----
        kwt_ps = ps_pre.tile([128, 2048], F32, tag="kwt")
        for ab in range(4):
            for hc in range(4):
                nc.tensor.matmul(
                    kwt_ps[:, ab * 512:(ab + 1) * 512],
                    w1bB[:, hc * A + ab * 128: hc * A + (ab + 1) * 128],
                    kTs[:, hc * 512:(hc + 1) * 512],
                    start=(hc == 0), stop=False)
            nc.tensor.matmul(
                kwt_ps[:, ab * 512:(ab + 1) * 512],
                b1r[:, ab * 128:(ab + 1) * 128],
                onesr[:],
                start=False, stop=True)

        # ---- atom emission helper ---------------------------------------
        def emit_atom(src, spec, out, clip_cache, width):
            kind = spec[0]
            if kind == 'sin':
                w = spec[1]
                c = _trig_clip(w)
                if c is not None:
                    ckey = ('clip', w)
                    if ckey not in clip_cache:
                        ct = sb.tile([128, width], F32,
                                     tag=f"clip{width}_{len(clip_cache)}")
                        nc.vector.tensor_scalar(ct[:], src[:], float(c),
                                                float(-c), OP.min, OP.max)
                        clip_cache[ckey] = ct
                    src = clip_cache[ckey]
                col = CONSTS_COLS[('s', spec[2])]
                nc.scalar.activation(out[:], src[:], AF.Sin,
                                     bias=cb[:, col:col + 1], scale=float(w))
            else:
                col = CONSTS_COLS[(kind, spec[1], spec[2])]
                nc.scalar.activation(out[:], src[:], AF_OF[kind],
                                     bias=cb[:, col:col + 1],
                                     scale=float(spec[1]))

        # ---- x-side atoms (tanh-family first, sins after clip) ----------
        xph = {}
        xclip_cache = {}
        x_order = ([i for i in USED_X if XATOMS[i][0] not in ('one', 'sin')] +
                   [i for i in USED_X if XATOMS[i][0] == 'sin'])
        for i in x_order:
            t = sb.tile([128, 512], BF16, tag=f"xf{i}")
            emit_atom(qwt_ps, XATOMS[i], t, xclip_cache, 512)
            xph[i] = t

        # ---- y lin atom (DVE cast from PSUM) ----------------------------
        yt = {}
        yclip_cache = {}
        for j in Y_LIN:
            t = sb.tile([128, 2048], BF16, tag=f"yf{j}")
            nc.vector.tensor_copy(t[:], kwt_ps[:])
            yt[j] = t

        # ---- y tanh-family atoms (ACT from PSUM) ------------------------
        for j in Y_TANH:
            t = sb.tile([128, 2048], BF16, tag=f"yf{j}")
            emit_atom(kwt_ps, YATOMS[j], t, yclip_cache, 2048)
            yt[j] = t

        # ---- F_j chains on DVE: raw-atom combo, then w2 broadcast fold --
        fts = {}
        for n, j in enumerate(Y_ORDER):
            terms = [(i, c) for (i, c) in YGROUPS[j] if XATOMS[i][0] != 'one']
            ones = [c for (i, c) in YGROUPS[j] if XATOMS[i][0] == 'one']
            c_one = float(sum(ones))
            ft = sb.tile([128, 512], BF16, tag=f"F{j}")
            if terms:
                acc = sb.tile([128, 512], BF16, tag=f"Fa{j}")
                i0, c0 = terms[0]
                nc.vector.tensor_scalar_mul(acc[:], xph[i0][:], float(c0))
                for (ii, cc) in terms[1:]:
                    nc.vector.scalar_tensor_tensor(acc[:], xph[ii][:],
                                                   float(cc), acc[:],
                                                   OP.mult, OP.add)
                if ones:
                    nc.vector.scalar_tensor_tensor(ft[:], acc[:], c_one,
                                                   w2bc[:], OP.add, OP.mult)
                else:
                    nc.vector.tensor_mul(ft[:], acc[:], w2bc[:])
            else:
                nc.vector.tensor_scalar_mul(ft[:], w2bc[:], c_one)
            fts[j] = ft

        # ---- y sin atoms (clip on DVE; ACT reads clipped SBUF) ----------
        for j in Y_SIN:
            t = sb.tile([128, 2048], BF16, tag=f"yf{j}")
            emit_atom(kwt_ps, YATOMS[j], t, yclip_cache, 2048)
            yt[j] = t

        # qwt/kwt PSUM banks are dead past this point; free them for tail
        pre_ctx.close()
        ps_tail = ctx.enter_context(tc.tile_pool(name="pstail", bufs=1,
                                                 space="PSUM"))
        ps_tp = ctx.enter_context(tc.tile_pool(name="pstp", bufs=2,
                                               space="PSUM"))

        # ---- score matmuls: per y-atom, k-half A then k-half B ----------
        sc_A_full = ps_sc.tile([128, 512], F32, tag="scA")
        sc_B_full = ps_sc.tile([128, 512], F32, tag="scB")
        sc_A = sc_A_full[:, 0:256]
        sc_B = sc_B_full[:, 0:256]
        nj = len(Y_ORDER)
        for n, j in enumerate(Y_ORDER):
            for half, sc in ((0, sc_A), (1, sc_B)):
                for ab in range(4):
                    nc.tensor.matmul(
                        sc[:],
                        fts[j][:, ab * 128:(ab + 1) * 128],
                        yt[j][:, ab * 512 + half * 256:
                              ab * 512 + half * 256 + 256],
                        start=(n == 0 and ab == 0),
                        stop=(n == nj - 1 and ab == 3))

        # exp table prefetch while the score matmuls drain (depends on the
        # last y atom so it schedules after all tanh/sin activations)
        dummy = sb.tile([128, 1], F32, tag="dummy")
        nc.scalar.activation(dummy[:], yt[Y_ORDER[-1]][:, 0:1], AF.Exp,
                             bias=0.0, scale=1.0)

        # ---- per-half masked softmax + context --------------------------
        ctx_ps = ps_tail.tile([128, 512], F32, tag="ctx")
        ssum = {}
        wexp = {}
        for half, sc in ((0, sc_A), (1, sc_B)):
            sm = sb.tile([128, 256], F32, tag=f"sm{half}")
            nc.vector.scalar_tensor_tensor(sm[:], mf[:, half * 256:
                                                     half * 256 + 256],
                                           MASK_NEG, sc[:], OP.mult, OP.add)
            we = sb.tile([128, 256], BF16, tag=f"we{half}")
            ss = sb.tile([128, 1], F32, tag=f"ss{half}")
            nc.scalar.activation(we[:], sm[:], AF.Exp, bias=0.0, scale=1.0,
                                 accum_out=ss[:])
            wexp[half] = we
            ssum[half] = ss
            wT = sb.tile([128, 256], BF16, tag=f"wT{half}")
            for i in range(2):
                pt = ps_tp.tile([128, 128], BF16, tag="tp")
                nc.tensor.transpose(pt[:], we[:, i * 128:(i + 1) * 128],
                                    ident[:])
                nc.vector.tensor_copy(wT[:, i * 128:(i + 1) * 128], pt[:])
            for i in range(2):
                kc = half * 2 + i
                nc.tensor.matmul(ctx_ps[:], wT[:, i * 128:(i + 1) * 128],
                                 vb[:, kc * 512:(kc + 1) * 512],
                                 start=(kc == 0), stop=(kc == 3))

        stot = sb.tile([128, 1], F32, tag="stot")
        nc.vector.tensor_add(stot[:], ssum[0][:], ssum[1][:])
        rinv = sb.tile([128, 1], F32, tag="rinv")
        nc.vector.reciprocal(rinv[:], stot[:])
        wout = sb.tile([128, 512], F32, tag="wout")
        for half in (0, 1):
            nc.vector.tensor_scalar_mul(wout[:, half * 256:half * 256 + 256],
                                        wexp[half][:], rinv[:])
            nc.sync.dma_start(d_wout[:, half * 256:half * 256 + 256],
                              wout[:, half * 256:half * 256 + 256])
        cout = sb.tile([128, 512], F32, tag="cout")
        nc.vector.tensor_scalar_mul(cout[:], ctx_ps[:], rinv[:])
        nc.sync.dma_start(d_cout[:], cout[:])

    nc.compile()
    return nc


_NC_CACHE = None


def _get_nc():
    global _NC_CACHE
    if _NC_CACHE is None:
        _NC_CACHE = _build_kernel()
    return _NC_CACHE


def _host_inputs(query, keys, values, mask, W1, b1, w2, b2):
    query = np.asarray(query, np.float32).astype(NPBF)
    keys = np.asarray(keys, np.float32).astype(NPBF)
    values = np.asarray(values, np.float32).astype(NPBF)
    maskb = np.asarray(mask).astype(NPBF)
    W1 = np.ascontiguousarray(np.asarray(W1, np.float32).astype(NPBF))
    b1 = np.asarray(b1, np.float32)
    w2 = np.asarray(w2, np.float32)
    b1r = np.ascontiguousarray(b1.astype(NPBF).reshape(1, A))
    onesr = np.ones((1, K), dtype=NPBF)
    w2cc = np.ascontiguousarray(w2.reshape(4, 128).T.astype(np.float32))
    w2bc = np.ascontiguousarray(
        np.repeat(w2cc.astype(NPBF)[:, :, None], 128, axis=2).reshape(128, 512))
    consts = np.zeros((128, NCONSTS), np.float32)
    for c, v in enumerate(CONSTS_VALS):
        consts[:, c] = v
    ident = np.eye(128, dtype=NPBF)

    in_maps = []
    for c in range(N_CORES):
        b, qh = c // 2, c % 2
        in_maps.append({
            "qt": np.ascontiguousarray(query[b, qh * QSH:(qh + 1) * QSH, :].T),
            "kt": np.ascontiguousarray(keys[b].T),
            "v": np.ascontiguousarray(values[b]),
            "m": np.ascontiguousarray(maskb[b, qh * QSH:(qh + 1) * QSH, :]),
            "w1": W1,
            "b1r": b1r,
            "onesr": onesr,
            "w2bc": w2bc,
            "consts": consts,
            "ident": ident,
        })
    return in_maps


def _run(inputs, trace=False, **kw):
    nc = _get_nc()
    in_maps = _host_inputs(**inputs)
    res = run_bass_kernel_spmd(nc, in_maps, list(range(N_CORES)),
                               trace=trace, **kw)
    context = np.zeros((B, Q, H), np.float32)
    weights = np.zeros((B, Q, K), np.float32)
    for c in range(N_CORES):
        b, qh = c // 2, c % 2
        weights[b, qh * QSH:(qh + 1) * QSH, :] = res.results[c]["wout"]
        context[b, qh * QSH:(qh + 1) * QSH, :] = res.results[c]["cout"]
    return (context, weights), res


def kernel(query, keys, values, mask, W1, b1, w2, b2):
    (context, weights), _ = _run(dict(query=query, keys=keys, values=values,
                                      mask=mask, W1=W1, b1=b1, w2=w2, b2=b2))
    return context, weights


# revision 14
# speedup vs baseline: 2.5433x; 1.2057x over previous
"""Bahdanau attention kernel for 8 TRN2 NeuronCores.

Math: scores[q,k] = w2 . tanh(qW[q,:] + kW[k,:] + b1) (+ b2, dropped: softmax
is shift-invariant). The tanh over the [B,Q,K,A] tensor is replaced by a
separable product expansion fitted offline:

    tanh(x + y) ~= sum_j F_j(x) * psi_j(y),   F_j = w2 * sum_i C_ij phi_i(x)

The x-side combined functions F_j are folded on DVE (one op per nonzero C
entry + one w2-broadcast multiply) so the TensorEngine runs one contraction
group per y-function instead of one per (i,j) pair. b1 is folded into the kW
matmul as an extra rank-1 contraction chunk. Factor activations read qW/kW
straight from PSUM; tanh+sin live in one HW activation table
(silu_and_others) so there is a single table load (+1 for the final exp,
prefetched under the score matmuls). Softmax runs per k-half so
exp/mask/transpose/context overlap the score-matmul tail; masking is a -30
additive pre-exp term; no max subtraction (scores are bounded).

Sharding: data-parallel, core = (batch b, query-half qh); each core computes
a [128, 512] block of weights and context. Output: (context, weights).
"""

import numpy as np
import ml_dtypes

from contextlib import ExitStack
from concourse import bass, bacc, tile, mybir
from concourse.bass_utils import run_bass_kernel_spmd

BF16 = mybir.dt.bfloat16
F32 = mybir.dt.float32
AF = mybir.ActivationFunctionType
OP = mybir.AluOpType
NPBF = ml_dtypes.bfloat16

B, Q, K, H, A = 4, 256, 512, 512, 512
QSH = 128
N_CORES = 8
PH = float(np.pi / 4)
TMAX = 3.2          # |spline arg| budget for Sin
ALPHA = 1.5
MASK_NEG = -30.0

# ---- factor model (fitted offline; see fit.py / fit_run.py) ---------------
# atom spec: ('one',) | ('lin',) | ('tanh', a, mu) | ('sin', w, sgn)
#            | ('silu', a, mu) | ('relu', a, mu) | ('square', a, mu)
# J=4 tanh-only y-atoms, 5 used x-atoms (tanh/square), 10 nonzero C;
# everything (atoms + final exp) lives in one HW act table -> zero swaps.
# Fitted against measured HW activation profiles, validated end-to-end
# in numpy (incl bf16 fold effects): weights 4.4e-3 / context 5.0e-3.
XATOMS = [('one',), ('tanh', 2.0, 0.0), ('square', 1.0, 0.0),
          ('tanh', 2.4, 0.3), ('tanh', 2.4, -0.6), ('tanh', 2.4, 0.9),
          ('tanh', 1.6, -1.2), ('tanh', 1.3, 1.5), ('tanh', 1.0, 0.9),
          ('tanh', 1.3, -0.3), ('tanh', 1.0, -1.5)]
YATOMS = [('tanh', 1.0, 0.0), ('tanh', 1.0, 0.3), ('tanh', 1.0, -0.6),
          ('tanh', 1.0, 0.9)]
PAIRS = [
    (1, 0, 1.819892356993005),
    (1, 1, -2.073488201726385),
    (6, 1, 0.3056980786972892),
    (7, 0, 3.852759219662744),
    (7, 1, -2.5907771182798047),
    (7, 2, -1.6263935632545834),
    (8, 0, -5.534705731425035),
    (8, 1, 3.372021281745387),
    (8, 2, 1.862747038411056),
    (9, 0, 0.5661563894530219),
]
XMAX = 2.16
N_WARM = 48        # PE p-state warm-up matmuls bridging the DMA prologue


def _trig_clip(w):
    c = (TMAX - PH) / w
    return c if c < XMAX else None


def _consts_layout():
    cols = {('z',): 0}
    vals = [0.0]
    for spec in XATOMS + YATOMS:
        key = None
        bias = None
        if spec[0] in ('tanh', 'silu', 'relu', 'square'):
            key = (spec[0], spec[1], spec[2])
            bias = -spec[1] * spec[2]
        elif spec[0] == 'sin':
            key = ('s', spec[2])
            bias = PH * spec[2]
        if key is not None and key not in cols:
            cols[key] = len(vals)
            vals.append(bias)
    return cols, vals


CONSTS_COLS, CONSTS_VALS = _consts_layout()
NCONSTS = len(CONSTS_VALS)

AF_OF = {'tanh': AF.Tanh, 'sin': AF.Sin, 'silu': AF.Silu, 'relu': AF.Relu,
         'square': AF.Square}

# group pairs by y-atom: j -> [(i, c), ...]; 'one' terms handled at fold time
YGROUPS = {}
for (xi, yi, cf) in PAIRS:
    YGROUPS.setdefault(yi, []).append((xi, cf))
for yi in YGROUPS:
    YGROUPS[yi].sort(key=lambda t: t[0])
USED_X = sorted({p[0] for p in PAIRS})
Y_TANH = [j for j, s in enumerate(YATOMS)
          if s[0] in ('tanh', 'silu', 'relu', 'square') and j in YGROUPS]
Y_SIN = [j for j, s in enumerate(YATOMS) if s[0] == 'sin' and j in YGROUPS]
Y_LIN = [j for j, s in enumerate(YATOMS) if s[0] == 'lin' and j in YGROUPS]
if Y_TANH:
    Y_ORDER = [Y_TANH[0]] + Y_LIN + Y_TANH[1:] + Y_SIN
else:
    Y_ORDER = Y_LIN + Y_SIN


def _build_kernel():
    nc = bacc.Bacc("TRN2", target_bir_lowering=False, debug=False,
                   num_devices=N_CORES)

    d_qt = nc.declare_dram_parameter("qt", [H, QSH], BF16, isOutput=False)
    d_kt = nc.declare_dram_parameter("kt", [H, K], BF16, isOutput=False)
    d_v = nc.declare_dram_parameter("v", [K, H], BF16, isOutput=False)
    d_m = nc.declare_dram_parameter("m", [QSH, K], BF16, isOutput=False)
    d_w1 = nc.declare_dram_parameter("w1", [2 * H, A], BF16, isOutput=False)
    d_b1r = nc.declare_dram_parameter("b1r", [1, A], BF16, isOutput=False)
    d_ones = nc.declare_dram_parameter("onesr", [1, K], BF16, isOutput=False)
    d_w2bc = nc.declare_dram_parameter("w2bc", [128, 512], BF16, isOutput=False)
    d_cb = nc.declare_dram_parameter("consts", [128, NCONSTS], F32,
                                     isOutput=False)
    d_id = nc.declare_dram_parameter("ident", [128, 128], BF16, isOutput=False)
    d_wout = nc.declare_dram_parameter("wout", [QSH, K], F32, isOutput=True)
    d_cout = nc.declare_dram_parameter("cout", [QSH, H], F32, isOutput=True)

    with tile.TileContext(nc) as tc, ExitStack() as ctx:
        sb = ctx.enter_context(tc.tile_pool(name="sb", bufs=1))
        ps_sc = ctx.enter_context(tc.tile_pool(name="pssc", bufs=1,
                                               space="PSUM"))
        pre_ctx = ExitStack()
        ps_pre = pre_ctx.enter_context(tc.tile_pool(name="pspre", bufs=1,
                                                    space="PSUM"))
        ps_dum = pre_ctx.enter_context(tc.tile_pool(name="psdum", bufs=1,
                                                    space="PSUM"))

        # ---- PE p-state warm-up: keep the systolic array busy while the
        # input DMAs land so the real matmuls run at full clock -----------
        dum_in = sb.tile([128, 128], BF16, tag="dum_in")
        nc.vector.memset(dum_in[:], 0.0)
        dum_mv = sb.tile([128, 512], BF16, tag="dum_mv")
        nc.vector.memset(dum_mv[:], 0.0)
        dum_ps = ps_dum.tile([128, 512], F32, tag="dum_ps")
        for _ in range(N_WARM):
            nc.tensor.matmul(dum_ps[:], dum_in[:], dum_mv[:],
                             start=True, stop=True)

        # ---- input DMAs: one rearranged DMA per tensor ------------------
        # sync queue: qWT path first, then kWT path; gpsimd: the rest
        w1bA = sb.tile([128, 4 * A], BF16, tag="w1bA")
        w1bB = sb.tile([128, 4 * A], BF16, tag="w1bB")
        qTs = sb.tile([128, 4 * QSH], BF16, tag="qTs")
        kTs = sb.tile([128, 4 * K], BF16, tag="kTs")
        cb = sb.tile([128, NCONSTS], F32, tag="cb")
        nc.sync.dma_start(cb[:], d_cb[:])
        nc.sync.dma_start(w1bA[:].rearrange("p (hc c) -> p hc c", hc=4),
                          d_w1[0:512, :].rearrange("(hc p) c -> p hc c", hc=4))
        nc.sync.dma_start(qTs[:].rearrange("p (hc c) -> p hc c", hc=4),
                          d_qt[:].rearrange("(hc p) c -> p hc c", hc=4))
        nc.scalar.dma_start(w1bB[:].rearrange("p (hc c) -> p hc c", hc=4),
                            d_w1[512:1024, :].rearrange("(hc p) c -> p hc c",
                                                        hc=4))
        nc.gpsimd.dma_start(kTs[:].rearrange("p (hc c) -> p hc c", hc=4),
                            d_kt[:].rearrange("(hc p) c -> p hc c", hc=4))
        b1r = sb.tile([1, A], BF16, tag="b1r")
        nc.gpsimd.dma_start(b1r[:], d_b1r[:])
        onesr = sb.tile([1, K], BF16, tag="onesr")
        nc.gpsimd.dma_start(onesr[:], d_ones[:])
        w2bc = sb.tile([128, 512], BF16, tag="w2bc")
        nc.gpsimd.dma_start(w2bc[:], d_w2bc[:])
        vb = sb.tile([128, 4 * H], BF16, tag="vb")
        nc.gpsimd.dma_start(vb[:].rearrange("p (kc c) -> p kc c", kc=4),
                            d_v[:].rearrange("(kc p) c -> p kc c", kc=4))
        mf = sb.tile([128, K], BF16, tag="mf")
        nc.gpsimd.dma_start(mf[:], d_m[:])
        ident = sb.tile([128, 128], BF16, tag="ident")
        nc.gpsimd.dma_start(ident[:], d_id[:])

        # ---- qWT [a, q] in PSUM (ab-outer: sequential PSUM acc groups) --
        qwt_ps = ps_pre.tile([128, 512], F32, tag="qwt")
        for ab in range(4):
            for hc in range(4):
                nc.tensor.matmul(
                    qwt_ps[:, ab * 128:(ab + 1) * 128],
                    w1bA[:, hc * A + ab * 128: hc * A + (ab + 1) * 128],
                    qTs[:, hc * 128:(hc + 1) * 128],
                    start=(hc == 0), stop=(hc == 3))

        # ---- kWT [a, k] + b1 (extra rank-1 chunk closes each group) -----
        kwt_ps = ps_pre.tile([128, 2048], F32, tag="kwt")
        for ab in range(4):
            for hc in range(4):
                nc.tensor.matmul(
                    kwt_ps[:, ab * 512:(ab + 1) * 512],
                    w1bB[:, hc * A + ab * 128: hc * A + (ab + 1) * 128],
                    kTs[:, hc * 512:(hc + 1) * 512],
                    start=(hc == 0), stop=False)
            nc.tensor.matmul(
                kwt_ps[:, ab * 512:(ab + 1) * 512],
                b1r[:, ab * 128:(ab + 1) * 128],
                onesr[:],
                start=False, stop=True)

        # ---- atom emission helper ---------------------------------------
        def emit_atom(src, spec, out, clip_cache, width):
            kind = spec[0]
            if kind == 'sin':
                w = spec[1]
                c = _trig_clip(w)
                if c is not None:
                    ckey = ('clip', w)
                    if ckey not in clip_cache:
                        ct = sb.tile([128, width], F32,
                                     tag=f"clip{width}_{len(clip_cache)}")
                        nc.vector.tensor_scalar(ct[:], src[:], float(c),
                                                float(-c), OP.min, OP.max)
                        clip_cache[ckey] = ct
                    src = clip_cache[ckey]
                col = CONSTS_COLS[('s', spec[2])]
                nc.scalar.activation(out[:], src[:], AF.Sin,
                                     bias=cb[:, col:col + 1], scale=float(w))
            else:
                col = CONSTS_COLS[(kind, spec[1], spec[2])]
                nc.scalar.activation(out[:], src[:], AF_OF[kind],
                                     bias=cb[:, col:col + 1],
                                     scale=float(spec[1]))

        # ---- x-side atoms (tanh-family first, sins after clip) ----------
        xph = {}
        xclip_cache = {}
        x_order = ([i for i in USED_X if XATOMS[i][0] not in ('one', 'sin')] +
                   [i for i in USED_X if XATOMS[i][0] == 'sin'])
        for i in x_order:
            t = sb.tile([128, 512], BF16, tag=f"xf{i}")
            emit_atom(qwt_ps, XATOMS[i], t, xclip_cache, 512)
            xph[i] = t

        # ---- y lin atom (DVE cast from PSUM) ----------------------------
        yt = {}
        yclip_cache = {}
        for j in Y_LIN:
            t = sb.tile([128, 2048], BF16, tag=f"yf{j}")
            nc.vector.tensor_copy(t[:], kwt_ps[:])
            yt[j] = t

        # ---- y tanh-family atoms (ACT from PSUM) ------------------------
        for j in Y_TANH:
            t = sb.tile([128, 2048], BF16, tag=f"yf{j}")
            emit_atom(kwt_ps, YATOMS[j], t, yclip_cache, 2048)
            yt[j] = t

        # ---- F_j chains on DVE: raw-atom combo, then w2 broadcast fold --
        fts = {}
        for n, j in enumerate(Y_ORDER):
            terms = [(i, c) for (i, c) in YGROUPS[j] if XATOMS[i][0] != 'one']
            ones = [c for (i, c) in YGROUPS[j] if XATOMS[i][0] == 'one']
            c_one = float(sum(ones))
            ft = sb.tile([128, 512], BF16, tag=f"F{j}")
            if terms:
                acc = sb.tile([128, 512], BF16, tag=f"Fa{j}")
                i0, c0 = terms[0]
                nc.vector.tensor_scalar_mul(acc[:], xph[i0][:], float(c0))
                for (ii, cc) in terms[1:]:
                    nc.vector.scalar_tensor_tensor(acc[:], xph[ii][:],
                                                   float(cc), acc[:],
                                                   OP.mult, OP.add)
                if ones:
                    nc.vector.scalar_tensor_tensor(ft[:], acc[:], c_one,
                                                   w2bc[:], OP.add, OP.mult)
                else:
                    nc.vector.tensor_mul(ft[:], acc[:], w2bc[:])
            else:
                nc.vector.tensor_scalar_mul(ft[:], w2bc[:], c_one)
            fts[j] = ft

        # ---- y sin atoms (clip on DVE; ACT reads clipped SBUF) ----------
        for j in Y_SIN:
            t = sb.tile([128, 2048], BF16, tag=f"yf{j}")
            emit_atom(kwt_ps, YATOMS[j], t, yclip_cache, 2048)
            yt[j] = t

        # qwt/kwt PSUM banks are dead past this point; free them for tail
        pre_ctx.close()
        ps_tail = ctx.enter_context(tc.tile_pool(name="pstail", bufs=1,
                                                 space="PSUM"))
        ps_tp = ctx.enter_context(tc.tile_pool(name="pstp", bufs=2,
                                               space="PSUM"))

        # ---- score matmuls: per y-atom, k-half A then k-half B ----------
        sc_A_full = ps_sc.tile([128, 512], F32, tag="scA")
        sc_B_full = ps_sc.tile([128, 512], F32, tag="scB")
        sc_A = sc_A_full[:, 0:256]
        sc_B = sc_B_full[:, 0:256]
        nj = len(Y_ORDER)
        for n, j in enumerate(Y_ORDER):
            for half, sc in ((0, sc_A), (1, sc_B)):
                for ab in range(4):
                    nc.tensor.matmul(
                        sc[:],
                        fts[j][:, ab * 128:(ab + 1) * 128],
                        yt[j][:, ab * 512 + half * 256:
                              ab * 512 + half * 256 + 256],
                        start=(n == 0 and ab == 0),
                        stop=(n == nj - 1 and ab == 3))

        # exp table prefetch while the score matmuls drain (depends on the
        # last y atom so it schedules after all tanh/sin activations)
        dummy = sb.tile([128, 1], F32, tag="dummy")
        nc.scalar.activation(dummy[:], yt[Y_ORDER[-1]][:, 0:1], AF.Exp,
                             bias=0.0, scale=1.0)

        # ---- per-half masked softmax + context --------------------------
        ctx_ps = ps_tail.tile([128, 512], F32, tag="ctx")
        ssum = {}
        wexp = {}
        for half, sc in ((0, sc_A), (1, sc_B)):
            sm = sb.tile([128, 256], F32, tag=f"sm{half}")
            nc.vector.scalar_tensor_tensor(sm[:], mf[:, half * 256:
                                                     half * 256 + 256],
                                           MASK_NEG, sc[:], OP.mult, OP.add)
            we = sb.tile([128, 256], BF16, tag=f"we{half}")
            ss = sb.tile([128, 1], F32, tag=f"ss{half}")
            nc.scalar.activation(we[:], sm[:], AF.Exp, bias=0.0, scale=1.0,
                                 accum_out=ss[:])
            wexp[half] = we
            ssum[half] = ss
            wT = sb.tile([128, 256], BF16, tag=f"wT{half}")
            for i in range(2):
                pt = ps_tp.tile([128, 128], BF16, tag="tp")
                nc.tensor.transpose(pt[:], we[:, i * 128:(i + 1) * 128],
                                    ident[:])
                nc.vector.tensor_copy(wT[:, i * 128:(i + 1) * 128], pt[:])
            for i in range(2):
                kc = half * 2 + i
                nc.tensor.matmul(ctx_ps[:], wT[:, i * 128:(i + 1) * 128],
                                 vb[:, kc * 512:(kc + 1) * 512],
                                 start=(kc == 0), stop=(kc == 3))

        stot = sb.tile([128, 1], F32, tag="stot")
        nc.vector.tensor_add(stot[:], ssum[0][:], ssum[1][:])
        rinv = sb.tile([128, 1], F32, tag="rinv")
        nc.vector.reciprocal(rinv[:], stot[:])
        wout = sb.tile([128, 512], F32, tag="wout")
        for half in (0, 1):
            nc.vector.tensor_scalar_mul(wout[:, half * 256:half * 256 + 256],
                                        wexp[half][:], rinv[:])
            nc.sync.dma_start(d_wout[:, half * 256:half * 256 + 256],
                              wout[:, half * 256:half * 256 + 256])
        cout = sb.tile([128, 512], F32, tag="cout")
        nc.vector.tensor_scalar_mul(cout[:], ctx_ps[:], rinv[:])
        nc.sync.dma_start(d_cout[:], cout[:])

    nc.compile()
    return nc


_NC_CACHE = None


def _get_nc():
    global _NC_CACHE
    if _NC_CACHE is None:
        _NC_CACHE = _build_kernel()
    return _NC_CACHE


def _host_inputs(query, keys, values, mask, W1, b1, w2, b2):
    query = np.asarray(query, np.float32).astype(NPBF)
    keys = np.asarray(keys, np.float32).astype(NPBF)
    values = np.asarray(values, np.float32).astype(NPBF)
    maskb = np.asarray(mask).astype(NPBF)
    W1 = np.ascontiguousarray(np.asarray(W1, np.float32).astype(NPBF))
    b1 = np.asarray(b1, np.float32)
    w2 = np.asarray(w2, np.float32)
    b1r = np.ascontiguousarray(b1.astype(NPBF).reshape(1, A))
    onesr = np.ones((1, K), dtype=NPBF)
    w2cc = np.ascontiguousarray(w2.reshape(4, 128).T.astype(np.float32))
    w2bc = np.ascontiguousarray(
        np.repeat(w2cc.astype(NPBF)[:, :, None], 128, axis=2).reshape(128, 512))
    consts = np.zeros((128, NCONSTS), np.float32)
    for c, v in enumerate(CONSTS_VALS):
        consts[:, c] = v
    ident = np.eye(128, dtype=NPBF)

    in_maps = []
    for c in range(N_CORES):
        b, qh = c // 2, c % 2
        in_maps.append({
            "qt": np.ascontiguousarray(query[b, qh * QSH:(qh + 1) * QSH, :].T),
            "kt": np.ascontiguousarray(keys[b].T),
            "v": np.ascontiguousarray(values[b]),
            "m": np.ascontiguousarray(maskb[b, qh * QSH:(qh + 1) * QSH, :]),
            "w1": W1,
            "b1r": b1r,
            "onesr": onesr,
            "w2bc": w2bc,
            "consts": consts,
            "ident": ident,
        })
    return in_maps


def _run(inputs, trace=False, **kw):
    nc = _get_nc()
    in_maps = _host_inputs(**inputs)
    res = run_bass_kernel_spmd(nc, in_maps, list(range(N_CORES)),
                               trace=trace, **kw)
    context = np.zeros((B, Q, H), np.float32)
    weights = np.zeros((B, Q, K), np.float32)
    for c in range(N_CORES):
        b, qh = c // 2, c % 2
        weights[b, qh * QSH:(qh + 1) * QSH, :] = res.results[c]["wout"]
        context[b, qh * QSH:(qh + 1) * QSH, :] = res.results[c]["cout"]
    return (context, weights), res


def kernel(query, keys, values, mask, W1, b1, w2, b2):
    (context, weights), _ = _run(dict(query=query, keys=keys, values=values,
                                      mask=mask, W1=W1, b1=b1, w2=w2, b2=b2))
    return context, weights
